# revision 82
# baseline (speedup 1.0000x reference)
"""Trainium2 Bass kernel for a dense transformer block (B=4,T=1024,H=1024,NH=16,FF=4096).

Sharding: 8 cores = (batch b, token-half h). Each core computes the full block
for its 512 query tokens; K/V projections are computed over all 1024 tokens of
the batch on each core (no cross-core collectives).

v2: fp8e4 DoubleRow matmuls for Q/K/V, AV+denominator and O-projection
(weights scaled x1024 host-side; descale folded into correction ops), plus
LayerNorm folding: K/Q projections contract over RAW x so they start
immediately; the LN affine is applied post-matmul per token as
q = rstd*(W'x) + (-mu*rstd)*u + c with u = colsum(W'), c = W^T ln_b + b,
overlapping the LN stats matmuls with the projection matmuls. Scores and the
MLP (fc/proj) stay bf16 for precision. Device layout is fully "transposed":
activations live as [feature->partitions, token->free] SBUF tiles; reductions
over features/keys are TensorE ones-matmuls. The reference's
softmax-then-multiplicative-mask semantics are kept: exp over all keys feeds
the denominator, masked exp feeds the AV matmul.
"""
import sys
sys.path.insert(0, "/opt/trn_rl_repo")
import numpy as np
import ml_dtypes

B, T, H, NH = 4, 1024, 1024, 16
HS = H // NH          # 64
FF = 4 * H            # 4096
EPS = 1e-5
P = 128
TQ = T // 2           # 512 query tokens per core
NT = T // P           # 8 feature/token tiles
NFF = FF // P         # 32
NCORES = 8
SW = 1024.0           # fp8 weight scale (power of 2)
ISW = 1.0 / SW

_bf16 = ml_dtypes.bfloat16
_f8 = ml_dtypes.float8_e4m3


# ----------------------------------------------------------------------------
# device program
# ----------------------------------------------------------------------------

def build(repeat=1, debug_outputs=False, c0=False, bo0=False, cv0=False,
          rope_dma=True, skip_mlp=False, skip_attn=False, empty_body=False,
          upto=0, v_nodr=False, v_not2v=False, av_nodr=False):
    import concourse.bass as bass
    import concourse.mybir as mybir
    import concourse.tile as tile
    from concourse import bacc
    from contextlib import ExitStack

    f32 = mybir.dt.float32
    bf = mybir.dt.bfloat16
    f8 = mybir.dt.float8e4
    AF = mybir.ActivationFunctionType
    ALU = mybir.AluOpType
    DR = mybir.MatmulPerfMode.DoubleRow

    nc = bacc.Bacc("TRN2", target_bir_lowering=False, debug=False,
                   num_devices=NCORES)

    def din(name, shape, dt=f32):
        return nc.dram_tensor(name, shape, dt, kind="ExternalInput").ap()

    # per-core inputs
    xT_lo = din("xT_lo", [H, TQ], bf)        # x^T columns 0:512 (bf16)
    xT_hi = din("xT_hi", [H, TQ], bf)        # x^T columns 512:1024 (= query tokens)
    xT8 = din("xT8", [H, T], f8)             # x^T full, fp8 (for K/Q matmuls)
    wq = din("wq", [H, H], f8)               # [h_in, f_out]*SW*ln1w, rope-permuted
    wk = din("wk", [H, H], f8)
    wv = din("wv", [H, H], f8)               # Wv*SW
    wo = din("wo", [H, H], f8)               # Wo*SW
    wfc = din("wfc", [H, FF], bf)
    wpr = din("wpr", [FF, H], bf)
    uq = din("uq", [P, NT])                  # colsum(W'q), permuted, col tiles
    uk = din("uk", [P, NT])
    cq = din("cq", [P, NT])                  # Wq^T ln1b + bq, permuted
    ck = din("ck", [P, NT])
    bo = din("bo", [P, NT])
    bpr = din("bpr", [P, NT])
    bfc = din("bfc", [P, NFF])
    ln2w = din("ln2w", [P, NT])
    ln2b = din("ln2b", [P, NT])
    uvb = din("uvb", [P, H], bf)             # colsum(W'v), broadcast across partitions
    cvb = din("cvb", [P, H])                 # Wv^T ln1b + bv, broadcast
    cosK = din("cosK", [P, T], bf)           # rope tables, xT column order
    ssgnK = din("ssgnK", [P, T], bf)         # +sin rows j=0, -sin rows j=1
    p32 = din("p32", [P, P], bf)             # partition-swap(32) permutation
    mscal = din("mscal", [P, 1])             # kt 0..3 mask as scalar (0.0 or 1.0)
    mask_hi = din("mask_hi", [P, 4, TQ], f8) # kt 4..7 triangular masks

    outT = nc.dram_tensor("outT", [H, TQ], f32, kind="ExternalOutput").ap()
    dbg = {}
    if debug_outputs:
        for name, shape in [("d_kT", [H, T]), ("d_qT", [H, TQ]),
                            ("d_v", [T, H]), ("d_attnT", [H, TQ]),
                            ("d_x2T", [H, TQ]), ("d_mT", [FF, TQ])]:
            dbg[name] = nc.dram_tensor(name, shape, f32, kind="ExternalOutput").ap()

    def body(tc, const1, ones_bf, ones_f8):
        if empty_body:
            with ExitStack() as ctx:
                pool = ctx.enter_context(tc.tile_pool(name="ep", bufs=2))
                psp = ctx.enter_context(tc.tile_pool(name="epp", bufs=1, space="PSUM"))
                xt = pool.tile([P, NT, TQ], bf, tag="xt")
                nc.sync.dma_start(out=xt, in_=xT_hi.rearrange("(n p) t -> p n t", p=P))
                ps = psp.tile([P, TQ], f32, tag="ps")
                nc.tensor.matmul(ps, ones_bf, xt[:, 0, :], start=True, stop=True)
                ot = pool.tile([P, TQ], f32, tag="ot")
                nc.vector.tensor_copy(ot, ps)
                nc.sync.dma_start(out=outT[0:P, :], in_=ot)
            return
        with ExitStack() as ctx:
            # ------------ long-lived pools for this block iteration ----------
            persist = ctx.enter_context(tc.tile_pool(name="persist", bufs=1))
            x_hi = persist.tile([P, NT, TQ], bf, tag="x_hi")
            x2T = persist.tile([P, NT, TQ], f32, tag="x2T")
            uc = persist.tile([P, 4 * NT], f32, tag="uc")  # uq|uk|cq|ck
            bo_sb = persist.tile([P, NT], f32, tag="bo_sb")
            bpr_sb = persist.tile([P, NT], f32, tag="bpr_sb")
            bfc_sb = persist.tile([P, NFF], f32, tag="bfc_sb")
            ln_sb = persist.tile([P, 2 * NT], f32, tag="ln_sb")  # ln2w|ln2b
            eps_sb = persist.tile([P, 1], f32, tag="eps_sb")
            nc.vector.memset(eps_sb, EPS)
            epsS_sb = persist.tile([P, 1], f32, tag="epsS_sb")
            nc.vector.memset(epsS_sb, EPS * SW * SW)
            msc_sb = persist.tile([P, 1], f32, tag="msc_sb")
            nc.sync.dma_start(out=msc_sb, in_=mscal)

            nc.sync.dma_start(out=uc[:, 0:NT], in_=uq)
            nc.sync.dma_start(out=uc[:, NT:2 * NT], in_=uk)
            nc.sync.dma_start(out=uc[:, 2 * NT:3 * NT], in_=cq)
            nc.sync.dma_start(out=uc[:, 3 * NT:4 * NT], in_=ck)
            nc.sync.dma_start(out=bo_sb, in_=bo)
            nc.sync.dma_start(out=bpr_sb, in_=bpr)
            nc.sync.dma_start(out=bfc_sb, in_=bfc)
            nc.sync.dma_start(out=ln_sb[:, 0:NT], in_=ln2w)
            nc.sync.dma_start(out=ln_sb[:, NT:2 * NT], in_=ln2b)
            nc.sync.dma_start(out=x_hi, in_=xT_hi.rearrange("(n p) t -> p n t", p=P))

            with ExitStack() as attn_scope:
                aacts = attn_scope.enter_context(tc.tile_pool(name="aacts", bufs=1))
                x8 = aacts.tile([P, NT, T], f8, tag="x8")
                kT = aacts.tile([P, NT, T], bf, tag="kT")
                qT = aacts.tile([P, NT, TQ], bf, tag="qT")
                vsb = aacts.tile([P, NT, H], f8, tag="vsb")     # [tok_p, kt, feat]
                attnT = aacts.tile([P, NT, TQ], f8, tag="attnT")
                # [V'|ones] fused stationary operand for kt 0..3
                # layout [p, kt, head, {V',ones}, hs]
                vaug = aacts.tile([P, 4, NH, 2, HS], f8, tag="vaug")
                # [V|0] stationary for the masked kt 4..7 AV accumulation
                vzer = aacts.tile([P, 4, NH, 2, HS], f8, tag="vzer")
                # [0|ones] shared stationary: kt 4..7 unmasked-denominator rows
                zo = aacts.tile([P, 2, 2, HS], f8, tag="zo")
                cos_sb = aacts.tile([P, T], bf, tag="cos_sb")
                ssgn_sb = aacts.tile([P, T], bf, tag="ssgn_sb")
                p32_sb = aacts.tile([P, P], bf, tag="p32_sb")
                nc.sync.dma_start(out=p32_sb, in_=p32)
                mhi_sb = aacts.tile([P, 4, TQ], f8, tag="mhi_sb")
                x_lo = aacts.tile([P, NT, TQ], bf, tag="x_lo")
                # stats tiles ([P,T] broadcast rows)
                mu_sb = aacts.tile([P, T], f32, tag="mu_sb")
                rstd_s = aacts.tile([P, T], bf, tag="rstd_s")   # rstd/SW
                m2 = aacts.tile([P, T], bf, tag="m2")           # -mu*rstd
                # token-partition columns of rstd_s / m2 (via PE transpose)
                rstdS_T = aacts.tile([P, NT], f32, tag="rstdS_T")
                m2T = aacts.tile([P, NT], f32, tag="m2T")

                nc.sync.dma_start(out=x8, in_=xT8.rearrange("(n p) t -> p n t", p=P))
                uvb_sb = aacts.tile([P, H], bf, tag="uvb_sb")
                nc.sync.dma_start(out=uvb_sb, in_=uvb)
                if not cv0:
                    cvb_sb = aacts.tile([P, H], f32, tag="cvb_sb")
                    nc.sync.dma_start(out=cvb_sb, in_=cvb)
                nc.sync.dma_start(out=cos_sb, in_=cosK)
                nc.sync.dma_start(out=ssgn_sb, in_=ssgnK)
                nc.sync.dma_start(out=mhi_sb, in_=mask_hi)
                nc.sync.dma_start(out=x_lo, in_=xT_lo.rearrange("(n p) t -> p n t", p=P))

                # -------- pipelined phase 1-3: stats / KQV+rope / attention ---
                with ExitStack() as p2:
                    stat = p2.enter_context(tc.tile_pool(name="stat", bufs=1))
                    tmpp = p2.enter_context(tc.tile_pool(name="ln_tmp", bufs=3))
                    wpool = p2.enter_context(tc.tile_pool(name="wqkv", bufs=3))
                    rtmp = p2.enter_context(tc.tile_pool(name="rtmp", bufs=3))
                    epool = p2.enter_context(tc.tile_pool(name="epool", bufs=2))
                    dpool = p2.enter_context(tc.tile_pool(name="dpool", bufs=2))
                    psln_scope = ExitStack()
                    psln = psln_scope.enter_context(
                        tc.tile_pool(name="psln", bufs=1, space="PSUM"))

                    # ---- LN1 stats (PE first: 32 reduce matmuls) ----
                    mu_ps = psln.tile([P, T], f32, tag="ln_mu")
                    sq_ps = psln.tile([P, T], f32, tag="ln_sq")
                    for kt in range(NT):
                        sq_lo = tmpp.tile([P, TQ], bf, tag="sq_lo")
                        nc.vector.tensor_mul(sq_lo, x_lo[:, kt, :], x_lo[:, kt, :])
                        sq_hi = tmpp.tile([P, TQ], bf, tag="sq_hi")
                        nc.vector.tensor_mul(sq_hi, x_hi[:, kt, :], x_hi[:, kt, :])
                        first, last = (kt == 0), (kt == NT - 1)
                        nc.tensor.matmul(mu_ps[:, 0:TQ], ones_bf, x_lo[:, kt, :],
                                         start=first, stop=last)
                        nc.tensor.matmul(mu_ps[:, TQ:T], ones_bf, x_hi[:, kt, :],
                                         start=first, stop=last)
                        nc.tensor.matmul(sq_ps[:, 0:TQ], ones_bf, sq_lo,
                                         start=first, stop=last)
                        nc.tensor.matmul(sq_ps[:, TQ:T], ones_bf, sq_hi,
                                         start=first, stop=last)

                    tmp1 = stat.tile([P, T], f32, tag="lntmp1")
                    nc.scalar.activation(mu_sb, mu_ps, AF.Copy, scale=1.0 / H)
                    nc.vector.tensor_mul(tmp1, mu_sb, mu_sb)
                    nc.vector.scalar_tensor_tensor(tmp1, sq_ps, 1.0 / H, tmp1,
                                                   ALU.mult, ALU.subtract)
                    # rstd_s = 1/sqrt((var+eps)*SW^2) = rstd/SW
                    nc.scalar.activation(tmp1, tmp1, AF.Sqrt, bias=epsS_sb,
                                         scale=SW * SW)
                    with nc.allow_low_precision(
                            reason="bf16 rstd/m2: 0.4%% roundoff, q/k are bf16 anyway"):
                        nc.vector.reciprocal(rstd_s, tmp1)
                    # m2 = -mu*rstd = (mu * -SW) * rstd_s
                    nc.vector.scalar_tensor_tensor(m2, mu_sb, -float(SW), rstd_s,
                                                   ALU.mult, ALU.mult)
                    # token-partition columns of rstd_s / m2: transpose each
                    # [128,128] block on the PE (identical rows -> any
                    # column permutation works, reuse p32 as rhs)
                    with nc.allow_low_precision(
                            reason="bf16 PE transpose of broadcast stat rows"):
                        for tt in range(NT):
                            csl = slice(tt * P, (tt + 1) * P)
                            trp = psln.tile([P, P], bf, tag="tr_ps", bufs=2,
                                            name=f"tr{tt}")
                            nc.tensor.matmul(trp, rstd_s[:, csl], p32_sb,
                                             start=True, stop=True, is_transpose=True)
                            nc.vector.tensor_copy(rstdS_T[:, tt:tt + 1], trp[:, 0:1])
                            trp2 = psln.tile([P, P], bf, tag="tr_ps", bufs=2,
                                             name=f"trm{tt}")
                            nc.tensor.matmul(trp2, m2[:, csl], p32_sb,
                                             start=True, stop=True, is_transpose=True)
                            nc.vector.tensor_copy(m2T[:, tt:tt + 1], trp2[:, 0:1])
                    psln_scope.close()

                    # PSUM pools for the pipelined region: 2+4+2 = 8 banks
                    psqkv = p2.enter_context(tc.tile_pool(name="psqkv", bufs=2, space="PSUM"))
                    ps_s = p2.enter_context(tc.tile_pool(name="ps_s", bufs=2, space="PSUM"))
                    ps_av = p2.enter_context(tc.tile_pool(name="ps_av", bufs=2, space="PSUM"))

                    wv_sb = wpool.tile([P, NT, H], f8, tag="wv_all", bufs=1)
                    nc.sync.dma_start(out=wv_sb, in_=wv.rearrange("(n p) m -> p n m", p=P))

                    e_tiles = {}
                    pend_adds = {}

                    def flush_adds(upto_fo):
                        for f in sorted(list(pend_adds)):
                            if f > upto_fo:
                                continue
                            for mode, t2, t1x, dst, nm in pend_adds.pop(f):
                                if mode == "dma":
                                    nc.vector.tensor_add(dst, t2, t1x)
                                else:
                                    # PE permutation swap, deferred one fo so
                                    # t1 is ready when this hits the PE head
                                    sw = psqkv.tile([P, TQ], f32, tag="qkv_ps",
                                                    name=f"sw{nm}")
                                    nc.tensor.matmul(sw, p32_sb, t1x,
                                                     start=True, stop=True)
                                    nc.vector.tensor_add(dst, t2, sw)

                    def emit_kq(fo):
                        pend = []
                        for which in (0, 1):    # 0 = K (all T), 1 = Q (hi half)
                            wdram = wk if which == 0 else wq
                            u_off = NT if which == 0 else 0
                            c_off = 3 * NT if which == 0 else 2 * NT
                            cols = slice(0, T) if which == 0 else slice(TQ, T)
                            nblk = (cols.stop - cols.start) // TQ
                            wt = wpool.tile([P, NT, P], f8, tag="wqkv",
                                            name=f"w{which}_{fo}")
                            nc.sync.dma_start(
                                out=wt,
                                in_=wdram[:, fo * P:(fo + 1) * P]
                                .rearrange("(n p) m -> p n m", p=P))
                            for blk in range(nblk):
                                sl = slice(cols.start + blk * TQ, cols.start + (blk + 1) * TQ)
                                osl = slice(blk * TQ, (blk + 1) * TQ)
                                ps = psqkv.tile([P, TQ], f32, tag="qkv_ps",
                                                name=f"qkp{which}_{fo}_{blk}")
                                for kp in range(NT // 2):
                                    nc.tensor.matmul(ps, wt[:, 2 * kp:2 * kp + 2, :],
                                                     x8[:, 2 * kp:2 * kp + 2, sl],
                                                     start=(kp == 0), stop=(kp == NT // 2 - 1),
                                                     perf_mode=DR)
                                # LN-fold correction: braw = ps*rstd_s + m2*u (+ c)
                                t0 = rtmp.tile([P, TQ], f32, tag="corr0",
                                               name=f"c0_{which}_{fo}_{blk}")
                                nc.vector.tensor_mul(t0, ps, rstd_s[:, sl])
                                braw = rtmp.tile([P, TQ], bf, tag="braw",
                                                 name=f"braw{which}_{fo}_{blk}")
                                if c0:
                                    nc.vector.scalar_tensor_tensor(
                                        braw, m2[:, sl], uc[:, u_off + fo:u_off + fo + 1],
                                        t0, ALU.mult, ALU.add)
                                else:
                                    nc.vector.scalar_tensor_tensor(
                                        t0, m2[:, sl], uc[:, u_off + fo:u_off + fo + 1],
                                        t0, ALU.mult, ALU.add)
                                    nc.scalar.activation(braw, t0, AF.Identity,
                                                         bias=uc[:, c_off + fo:c_off + fo + 1])
                                # rope: r = braw*cos + swap32(braw*ssgn); the
                                # swap is a PE permutation matmul (no DMA)
                                t1 = rtmp.tile([P, TQ], bf, tag="ropet1",
                                               name=f"t1_{which}_{fo}_{blk}", bufs=6)
                                t2 = rtmp.tile([P, TQ], bf, tag="ropet2",
                                               name=f"t2_{which}_{fo}_{blk}", bufs=6)
                                nc.vector.tensor_mul(t1, braw, ssgn_sb[:, sl])
                                nc.vector.tensor_mul(t2, braw, cos_sb[:, sl])
                                dst = kT[:, fo, osl] if which == 0 else qT[:, fo, osl]
                                pend.append((which, blk, t1, t2, dst))
                        for which, blk, t1, t2, dst in pend:
                            if rope_dma:
                                # 32-row swap via 4 strided SBUF DMAs; the
                                # final add is deferred so the DVE never
                                # stalls at queue head waiting for the DMA
                                t1s = rtmp.tile([P, TQ], bf, tag="ropet1s",
                                                name=f"t1s_{which}_{fo}_{blk}",
                                                bufs=6)
                                for q in range(4):
                                    src = q * 32 + (32 if q % 2 == 0 else -32)
                                    nc.sync.dma_start(
                                        out=t1s[q * 32:(q + 1) * 32, :],
                                        in_=t1[src:src + 32, :])
                                pend_adds.setdefault(fo, []).append(
                                    ("dma", t2, t1s, dst, f"{which}_{fo}_{blk}"))
                            else:
                                pend_adds.setdefault(fo, []).append(
                                    ("mm", t2, t1, dst, f"{which}_{fo}_{blk}"))

                    def emit_v():
                        # V folded like K/Q: mm on raw x8 (token layout), then
                        # v = rstd_tok*mm/SW + m2_tok*uv (+ cv)
                        for tt in range(NT):
                            t2v = rtmp.tile([P, H], bf, tag="t2v", name=f"t2v{tt}")
                            if v_not2v:
                                nc.vector.memset(t2v, 0.0)
                            elif cv0:
                                nc.vector.tensor_scalar_mul(t2v, uvb_sb,
                                                            m2T[:, tt:tt + 1])
                            else:
                                nc.vector.scalar_tensor_tensor(
                                    t2v, uvb_sb, m2T[:, tt:tt + 1], cvb_sb,
                                    ALU.mult, ALU.add)
                            for fb in range(2):
                                fsl = slice(fb * TQ, (fb + 1) * TQ)
                                ps = psqkv.tile([P, TQ], f32, tag="qkv_ps", name=f"vps{tt}_{fb}")
                                if v_nodr:
                                    for kt in range(NT):
                                        nc.tensor.matmul(
                                            ps, x8[:, kt, tt * P:(tt + 1) * P],
                                            wv_sb[:, kt, fsl],
                                            start=(kt == 0), stop=(kt == NT - 1))
                                else:
                                    for kp in range(NT // 2):
                                        nc.tensor.matmul(
                                            ps, x8[:, 2 * kp:2 * kp + 2, tt * P:(tt + 1) * P],
                                            wv_sb[:, 2 * kp:2 * kp + 2, fsl],
                                            start=(kp == 0), stop=(kp == NT // 2 - 1),
                                            perf_mode=DR)
                                nc.vector.scalar_tensor_tensor(
                                    vsb[:, tt, fsl], ps, rstdS_T[:, tt:tt + 1],
                                    t2v[:, fsl], ALU.mult, ALU.add)

                        nc.gpsimd.memset(vaug[:, :, :, 1, :], 1.0)
                        nc.gpsimd.memset(vzer[:, :, :, 1, :], 0.0)
                        nc.gpsimd.memset(zo[:, :, 0, :], 0.0)
                        nc.gpsimd.memset(zo[:, :, 1, :], 1.0)
                        for kt in range(4):
                            nc.vector.tensor_scalar_mul(
                                vaug[:, kt, :, 0, :],
                                vsb[:, kt, :].rearrange("p (h d) -> p h d", h=NH),
                                msc_sb)
                            nc.vector.tensor_copy(
                                vzer[:, kt, :, 0, :],
                                vsb[:, 4 + kt, :].rearrange("p (h d) -> p h d", h=NH))

                    def emit_scores(hp):
                        flush_adds(hp)
                        ft = hp
                        e_pair = epool.tile([P, NT, 2, TQ], f8, tag="e_pair", name=f"ep{hp}", bufs=2)
                        e_tiles[hp] = e_pair
                        for kt in range(NT):
                            ks = slice(kt * P, (kt + 1) * P)
                            sp = ps_s.tile([P, 2, TQ], f32, tag="sp", name=f"sp{hp}_{kt}")
                            nc.tensor.matmul(sp[:, 0, :], kT[0:HS, ft, ks],
                                             qT[0:HS, ft, :],
                                             start=True, stop=True,
                                             tile_position=(0, 0))
                            nc.tensor.matmul(sp[:, 1, :], kT[HS:P, ft, ks],
                                             qT[HS:P, ft, :],
                                             start=True, stop=True,
                                             tile_position=(64, 0))
                            nc.scalar.activation(e_pair[:, kt, :, :], sp, AF.Exp,
                                                 scale=1.0 / np.sqrt(HS))

                    em_tiles = {}

                    def emit_em(hp):
                        hd0, hd1 = 2 * hp, 2 * hp + 1
                        e_pair = e_tiles[hp]
                        ems = {}
                        for hd, j in ((hd0, 0), (hd1, 1)):
                            em = epool.tile([P, 4, TQ], f8, tag="em",
                                            name=f"em{hd}", bufs=4)
                            ems[hd] = em
                            for i in range(4):
                                nc.vector.tensor_mul(em[:, i, :],
                                                     e_pair[:, 4 + i, j, :],
                                                     mhi_sb[:, i, :])
                        em_tiles[hp] = ems

                    def emit_av(hp):
                        hd0, hd1 = 2 * hp, 2 * hp + 1
                        ft = hp
                        if hp not in em_tiles:
                            emit_em(hp)
                        e_pair = e_tiles.pop(hp)
                        ems = em_tiles.pop(hp)
                        av2s = {}
                        for hd, j in ((hd0, 0), (hd1, 1)):
                            av2 = ps_av.tile([P, TQ], f32, tag="av2", name=f"av2_{hd}")
                            av2s[hd] = av2
                            # kt 0..3 [V'|ones] + kt 4..7 [0|ones]:
                            # rows 0:64 AV (kt<4), rows 64:128 full D
                            if av_nodr:
                                for kt in range(4):
                                    nc.tensor.matmul(av2,
                                                     vaug[:, kt, hd, :, :],
                                                     e_pair[:, kt, j, :],
                                                     start=(kt == 0), stop=False,
                                                     skip_group_check=True)
                                for kt in range(4, NT):
                                    nc.tensor.matmul(av2, zo[:, kt % 2, :, :],
                                                     e_pair[:, kt, j, :],
                                                     start=False, stop=False,
                                                     skip_group_check=True)
                            else:
                                for kp in range(2):
                                    nc.tensor.matmul(av2,
                                                     vaug[:, 2 * kp:2 * kp + 2, hd, :, :],
                                                     e_pair[:, 2 * kp:2 * kp + 2, j, :],
                                                     start=(kp == 0), stop=False,
                                                     perf_mode=DR,
                                                     skip_group_check=True)
                                for kp in range(2, NT // 2):
                                    nc.tensor.matmul(av2, zo,
                                                     e_pair[:, 2 * kp:2 * kp + 2, j, :],
                                                     start=False, stop=False,
                                                     perf_mode=DR,
                                                     skip_group_check=True)
                        for hd, j in ((hd0, 0), (hd1, 1)):
                            av2, em = av2s[hd], ems[hd]
                            # kt 4..7 masked AV via [V|0] stationary
                            if av_nodr:
                                for i in range(4):
                                    nc.tensor.matmul(av2,
                                                     vzer[:, i, hd, :, :],
                                                     em[:, i, :],
                                                     start=False, stop=(i == 3),
                                                     skip_group_check=True)
                            else:
                                for i in range(2):
                                    nc.tensor.matmul(av2,
                                                     vzer[:, 2 * i:2 * i + 2, hd, :, :],
                                                     em[:, 2 * i:2 * i + 2, :],
                                                     start=False, stop=(i == 1),
                                                     perf_mode=DR,
                                                     skip_group_check=True)
                        for hd, j in ((hd0, 0), (hd1, 1)):
                            av2 = av2s[hd]
                            fp = (hd % 2) * HS
                            rec = dpool.tile([P, TQ], f32, tag="rec", name=f"rec{hd}")
                            nc.vector.reciprocal(rec[0:HS, :], av2[HS:P, :])
                            nc.vector.tensor_mul(attnT[fp:fp + HS, ft, :],
                                                 av2[0:HS, :], rec[0:HS, :])

                    # -------- software-pipelined emission --------
                    def early_out():
                        flush_adds(NT)
                        with ExitStack() as po:
                            op = po.enter_context(tc.tile_pool(name="eo", bufs=3))
                            for o in range(NT):
                                ot = op.tile([P, TQ], f32, tag="ot", name=f"eo{o}")
                                nc.vector.tensor_copy(ot, x_hi[:, o, :])
                                nc.sync.dma_start(out=outT[o * P:(o + 1) * P, :], in_=ot)

                    if upto == 1:          # stats + transposes only
                        early_out()
                        return
                    if upto == 2:          # + K/Q fo=0
                        emit_kq(0)
                        early_out()
                        return
                    if upto == 3:          # + V
                        emit_kq(0)
                        emit_v()
                        early_out()
                        return
                    if upto == 4:          # + all K/Q (no scores/attention)
                        for fo in range(NT):
                            emit_kq(fo)
                        emit_v()
                        early_out()
                        return
                    if upto == 5:          # + scores/exp (no AV)
                        for fo in range(NT):
                            emit_kq(fo)
                            if fo >= 1:
                                emit_scores(fo - 1)
                        emit_v()
                        emit_scores(NT - 1)
                        early_out()
                        return
                    if skip_attn:
                        emit_kq(0)
                        emit_v()
                        flush_adds(NT)
                        for kt in range(NT):
                            nc.vector.tensor_copy(
                                attnT[:, kt, :],
                                x_lo[:, kt, :])
                    else:
                        emit_kq(0)
                        emit_kq(1)
                        emit_scores(0)
                        emit_v()
                        for fo in range(2, NT):
                            emit_kq(fo)
                            emit_scores(fo - 1)
                            emit_av(fo - 2)
                        emit_scores(NT - 1)
                        emit_av(NT - 2)
                        emit_av(NT - 1)

                    if debug_outputs:
                        for nm, srcT, width in (("d_kT", kT, T), ("d_qT", qT, TQ),
                                                ("d_v", vsb, H)):
                            dT = dbg[nm].rearrange("(n p) t -> p n t", p=P)
                            for kt in range(NT):
                                c = rtmp.tile([P, T], f32, tag="dbgc2", name=f"c{nm}_{kt}", bufs=1)
                                nc.vector.tensor_copy(c[:, 0:width], srcT[:, kt, :])
                                nc.sync.dma_start(out=dT[:, kt, :], in_=c[:, 0:width])
                        dT = dbg["d_attnT"].rearrange("(n p) t -> p n t", p=P)
                        for kt in range(NT):
                            c = dpool.tile([P, TQ], f32, tag="dbgc3", name=f"ca{kt}")
                            nc.vector.tensor_copy(c, attnT[:, kt, :])
                            nc.sync.dma_start(out=dT[:, kt, :], in_=c)

                # ---------------- phase 4: O-projection + residual -----------
                with ExitStack() as p4:
                    wpool = p4.enter_context(tc.tile_pool(name="wo_pool", bufs=1))
                    otmp = p4.enter_context(tc.tile_pool(name="otmp", bufs=3))
                    ps_o = p4.enter_context(tc.tile_pool(name="ps_o", bufs=2, space="PSUM"))
                    wo_sb = wpool.tile([P, NT, H], f8, tag="wo_all")
                    nc.sync.dma_start(out=wo_sb,
                                      in_=wo.rearrange("(n p) m -> p n m", p=P))
                    for o in range(NT):
                        osl = slice(o * P, (o + 1) * P)
                        ps = ps_o.tile([P, TQ], f32, tag="o_ps")
                        for kp in range(NT // 2):
                            nc.tensor.matmul(ps, wo_sb[:, 2 * kp:2 * kp + 2, osl],
                                             attnT[:, 2 * kp:2 * kp + 2, :],
                                             start=(kp == 0), stop=(kp == NT // 2 - 1),
                                             perf_mode=DR)
                        if bo0:
                            nc.vector.scalar_tensor_tensor(
                                x2T[:, o, :], ps, ISW, x_hi[:, o, :],
                                ALU.mult, ALU.add)
                        else:
                            t = otmp.tile([P, TQ], f32, tag="o_t", name=f"ot{o}")
                            nc.scalar.activation(t, ps, AF.Identity, scale=ISW,
                                                 bias=bo_sb[:, o:o + 1])
                            nc.vector.tensor_add(x2T[:, o, :], t, x_hi[:, o, :])
                    if debug_outputs:
                        nc.sync.dma_start(out=dbg["d_x2T"].rearrange("(n p) t -> p n t", p=P), in_=x2T)

            if skip_mlp:
                with ExitStack() as pskip:
                    op = pskip.enter_context(tc.tile_pool(name="out_pool", bufs=3))
                    for o in range(NT):
                        ot = op.tile([P, TQ], f32, tag="ot")
                        nc.vector.tensor_copy(ot, x2T[:, o, :])
                        nc.sync.dma_start(out=outT[o * P:(o + 1) * P, :], in_=ot)
                return

            # ---------------- phase 5: LN2 + MLP ----------------------------
            with ExitStack() as p5:
                stat = p5.enter_context(tc.tile_pool(name="stat2", bufs=1))
                tmpp = p5.enter_context(tc.tile_pool(name="ln2_tmp", bufs=3))
                h2p = p5.enter_context(tc.tile_pool(name="h2p", bufs=1))
                mp = p5.enter_context(tc.tile_pool(name="m_pool", bufs=1))
                h2T = h2p.tile([P, NT, TQ], bf, tag="h2T")
                mT = mp.tile([P, NFF, TQ], bf, tag="mT")

                with ExitStack() as p5a:
                    psln2 = p5a.enter_context(tc.tile_pool(name="psln2", bufs=1, space="PSUM"))
                    mu_ps = psln2.tile([P, TQ], f32, tag="ln2_mu")
                    sq_ps = psln2.tile([P, TQ], f32, tag="ln2_sq")
                    for kt in range(NT):
                        xbf = tmpp.tile([P, TQ], bf, tag="x2bf")
                        nc.scalar.activation(xbf, x2T[:, kt, :], AF.Copy)
                        sqbf = tmpp.tile([P, TQ], bf, tag="sq2bf")
                        nc.vector.tensor_mul(sqbf, xbf, xbf)
                        nc.tensor.matmul(mu_ps, ones_bf, xbf,
                                         start=(kt == 0), stop=(kt == NT - 1))
                        nc.tensor.matmul(sq_ps, ones_bf, sqbf,
                                         start=(kt == 0), stop=(kt == NT - 1))
                    mu_sb = stat.tile([P, TQ], f32, tag="mu2_sb")
                    rstd = stat.tile([P, TQ], f32, tag="rstd2")
                    tmp1 = stat.tile([P, TQ], f32, tag="ln2tmp1")
                    nc.scalar.activation(mu_sb, mu_ps, AF.Copy, scale=1.0 / H)
                    nc.vector.tensor_mul(tmp1, mu_sb, mu_sb)
                    nc.vector.scalar_tensor_tensor(tmp1, sq_ps, 1.0 / H, tmp1,
                                                   ALU.mult, ALU.subtract)
                    nc.scalar.activation(tmp1, tmp1, AF.Sqrt, bias=eps_sb)
                    nc.vector.reciprocal(rstd, tmp1)
                    for kt in range(NT):
                        t = tmpp.tile([P, TQ], f32, tag="h2tmp")
                        nc.vector.tensor_sub(t, x2T[:, kt, :], mu_sb)
                        nc.vector.tensor_mul(t, t, rstd)
                        nc.scalar.activation(h2T[:, kt, :], t, AF.Identity,
                                             bias=ln_sb[:, NT + kt:NT + kt + 1],
                                             scale=ln_sb[:, kt:kt + 1])

                # ---- phase 6+7: fc+gelu interleaved with pr pass 1 (o 0..3),
                # then pr pass 2 (o 4..7) over the finished mT ---------------
                with ExitStack() as p6:
                    wpool = p6.enter_context(tc.tile_pool(name="wfc_pool", bufs=2))
                    wpool2 = p6.enter_context(tc.tile_pool(name="wpr_pool", bufs=3))
                    op = p6.enter_context(tc.tile_pool(name="out_pool", bufs=3))
                    psfc = p6.enter_context(tc.tile_pool(name="psfc", bufs=3, space="PSUM"))
                    pspr = p6.enter_context(tc.tile_pool(name="pspr", bufs=1, space="PSUM"))
                    NO2 = NT // 2
                    pr_ps = [pspr.tile([P, TQ], f32, tag=f"pr_ps{o}", name=f"pr1_{o}")
                             for o in range(NO2)]
                    wpr_sb = []
                    for ffg in range(8):       # groups of 4 ff-tiles
                        wt = wpool.tile([P, NT, 4 * P], bf, tag="wfc_t")
                        nc.sync.dma_start(
                            out=wt,
                            in_=wfc[:, ffg * 4 * P:(ffg + 1) * 4 * P]
                            .rearrange("(n p) m -> p n m", p=P))
                        for fl in range(4):
                            ff = ffg * 4 + fl
                            ps = psfc.tile([P, TQ], f32, tag="fc_ps")
                            for kt in range(NT):
                                nc.tensor.matmul(
                                    ps, wt[:, kt, fl * P:(fl + 1) * P], h2T[:, kt, :],
                                    start=(kt == 0), stop=(kt == NT - 1))
                            nc.scalar.activation(mT[:, ff, :], ps, AF.Gelu,
                                                 bias=bfc_sb[:, ff:ff + 1])
                        # pr pass 1 for the 4 fk's of this group, o 0..3
                        wpt = wpool2.tile([P, 2, H], bf, tag="wpr_t",
                                          name=f"wpr{ffg}a")
                        nc.sync.dma_start(
                            out=wpt,
                            in_=wpr[ffg * 4 * P:(ffg * 4 + 2) * P, :]
                            .rearrange("(n p) m -> p n m", p=P))
                        wpt2 = wpool2.tile([P, 2, H], bf, tag="wpr_t",
                                           name=f"wpr{ffg}b")
                        nc.sync.dma_start(
                            out=wpt2,
                            in_=wpr[(ffg * 4 + 2) * P:(ffg * 4 + 4) * P, :]
                            .rearrange("(n p) m -> p n m", p=P))
                        for fl in range(4):
                            fk = ffg * 4 + fl
                            w = (wpt, wpt2)[fl // 2]
                            for o in range(NO2):
                                nc.tensor.matmul(
                                    pr_ps[o], w[:, fl % 2, o * P:(o + 1) * P],
                                    mT[:, fk, :],
                                    start=(fk == 0), stop=(fk == NFF - 1))
                    for o in range(NO2):
                        ot = op.tile([P, TQ], f32, tag="ot", name=f"ot1_{o}")
                        nc.vector.scalar_tensor_tensor(
                            ot, pr_ps[o], bpr_sb[:, o:o + 1], x2T[:, o, :],
                            ALU.add, ALU.add)
                        nc.sync.dma_start(
                            out=outT[o * P:(o + 1) * P, :], in_=ot)
                    # pass 2: o 4..7 (wpr re-streamed)
                    pr_ps2 = [pspr.tile([P, TQ], f32, tag=f"pr_ps{o - NO2}",
                                        name=f"pr2_{o}")
                              for o in range(NO2, NT)]
                    for fg in range(NFF // 2):
                        wt2 = wpool2.tile([P, 2, H], bf, tag="wpr_t",
                                          name=f"wpr2_{fg}")
                        nc.sync.dma_start(
                            out=wt2,
                            in_=wpr[fg * 2 * P:(fg + 1) * 2 * P, :]
                            .rearrange("(n p) m -> p n m", p=P))
                        for fl in range(2):
                            fk = 2 * fg + fl
                            for o in range(NO2, NT):
                                nc.tensor.matmul(
                                    pr_ps2[o - NO2], wt2[:, fl, o * P:(o + 1) * P],
                                    mT[:, fk, :],
                                    start=(fk == 0), stop=(fk == NFF - 1))
                    for o in range(NO2, NT):
                        ot = op.tile([P, TQ], f32, tag="ot", name=f"ot2_{o}")
                        nc.vector.scalar_tensor_tensor(
                            ot, pr_ps2[o - NO2], bpr_sb[:, o:o + 1], x2T[:, o, :],
                            ALU.add, ALU.add)
                        nc.sync.dma_start(
                            out=outT[o * P:(o + 1) * P, :], in_=ot)
                    if debug_outputs:
                        dT = dbg["d_mT"].rearrange("(n p) t -> p n t", p=P)
                        for ff in range(NFF):
                            c = wpool.tile([P, TQ], f32, tag="dbgc6")
                            nc.vector.tensor_copy(c, mT[:, ff, :])
                            nc.sync.dma_start(out=dT[:, ff, :], in_=c)

    with tile.TileContext(nc) as tc, ExitStack() as top:
        const1 = top.enter_context(tc.tile_pool(name="const1", bufs=1))
        ones_bf = const1.tile([P, P], bf)
        nc.vector.memset(ones_bf, 1.0)
        ones_f8 = const1.tile([P, 2, HS], f8)
        nc.vector.memset(ones_f8, 1.0)
        if repeat == 1:
            body(tc, const1, ones_bf, ones_f8)
        else:
            engs = (mybir.EngineType.PE, mybir.EngineType.DVE,
                    mybir.EngineType.Activation, mybir.EngineType.SP,
                    mybir.EngineType.Pool)
            with tc.For_i(0, repeat, 1, hint_engines=engs):
                body(tc, const1, ones_bf, ones_f8)

    nc.compile()
    return nc


# ----------------------------------------------------------------------------
# host-side input preparation
# ----------------------------------------------------------------------------

def _rope_tables():
    half = HS // 2
    inv_freq = 1.0 / (10000.0 ** (np.arange(half, dtype=np.float32) / half))
    t = np.arange(T, dtype=np.float32)
    ang = t[None, :] * inv_freq[(np.arange(P) % half)][:, None]   # [128, T]
    cos = np.cos(ang).astype(np.float32)
    sin = np.sin(ang).astype(np.float32)
    # ssgn rows: +sin for j=0 rows (p%64<32), -sin for j=1 rows
    sgn = np.where((np.arange(P) % HS) < half, 1.0, -1.0).astype(np.float32)
    ssgn = sin * sgn[:, None]
    return cos, ssgn


def _perm():
    # new pos (hd, j, i) <- old feature hd*64 + 2i + j
    idx = np.arange(H).reshape(NH, HS // 2, 2)
    return idx.transpose(0, 2, 1).reshape(H)


def _col_tiles(v):
    # [N] -> [128, N//128] with column j = v[j*128:(j+1)*128]
    return np.ascontiguousarray(v.reshape(-1, P).T).astype(np.float32)


def prepare_in_maps(inputs):
    x = np.asarray(inputs["x"], np.float32)
    deint = _perm()
    ln1w = np.asarray(inputs["ln1_w"], np.float32)
    ln1b = np.asarray(inputs["ln1_b"], np.float32)
    Wq = np.asarray(inputs["Wq"], np.float32)
    Wk = np.asarray(inputs["Wk"], np.float32)
    # folded weights: W' = diag(ln1w) @ W, columns rope-permuted, *SW, fp8
    Wq_f = (ln1w[:, None] * Wq)[:, deint]
    Wk_f = (ln1w[:, None] * Wk)[:, deint]
    wq_ = (Wq_f * SW).astype(_f8)
    wk_ = (Wk_f * SW).astype(_f8)
    Wv = np.asarray(inputs["Wv"], np.float32)
    wv_ = (ln1w[:, None] * Wv * SW).astype(_f8)
    wo_ = (np.asarray(inputs["Wo"], np.float32) * SW).astype(_f8)
    uv_ = wv_.astype(np.float32).sum(axis=0) * ISW
    cv_ = Wv.T @ ln1b + np.asarray(inputs["bv"], np.float32)
    wfc_ = np.asarray(inputs["Wfc"], np.float32).astype(_bf16)
    wpr_ = np.asarray(inputs["Wpr"], np.float32).astype(_bf16)
    # u = colsum of the *quantized* folded weight (exact correction), true scale
    uq_ = wq_.astype(np.float32).sum(axis=0) * ISW
    uk_ = wk_.astype(np.float32).sum(axis=0) * ISW
    # c = W^T ln1b + b (rope-permuted)
    cq_ = (Wq.T @ ln1b + np.asarray(inputs["bq"], np.float32))[deint]
    ck_ = (Wk.T @ ln1b + np.asarray(inputs["bk"], np.float32))[deint]
    cos, ssgn = _rope_tables()

    p32_ = (np.arange(P)[:, None] == (np.arange(P)[None, :] ^ 32)).astype(_bf16)

    ql = np.arange(TQ)
    mask_hi = np.zeros((P, 4, TQ), np.float32)
    for j in range(4):
        mask_hi[:, j, :] = (j * P + np.arange(P)[:, None]) <= ql[None, :]
    mask_hi = mask_hi.astype(_f8)

    shared = dict(
        wq=wq_, wk=wk_, wv=wv_, wo=wo_, wfc=wfc_, wpr=wpr_,
        uq=_col_tiles(uq_), uk=_col_tiles(uk_),
        cq=_col_tiles(cq_), ck=_col_tiles(ck_),
        bo=_col_tiles(np.asarray(inputs["bo"], np.float32)),
        bpr=_col_tiles(np.asarray(inputs["bpr"], np.float32)),
        bfc=_col_tiles(np.asarray(inputs["bfc"], np.float32)),
        ln2w=_col_tiles(np.asarray(inputs["ln2_w"], np.float32)),
        ln2b=_col_tiles(np.asarray(inputs["ln2_b"], np.float32)),
        uvb=np.broadcast_to(uv_[None, :], (P, H)).astype(_bf16),
        cvb=np.broadcast_to(cv_[None, :], (P, H)).astype(np.float32),
        mask_hi=mask_hi,
        p32=p32_,
    )

    in_maps = []
    for c in range(NCORES):
        b, h = c // 2, c % 2
        if h == 0:
            colperm = np.concatenate([np.arange(TQ, T), np.arange(0, TQ)])
        else:
            colperm = np.arange(T)
        xTb = np.ascontiguousarray(x[b].T[:, colperm])       # [H, T] rotated
        m = dict(shared)
        m["xT_lo"] = np.ascontiguousarray(xTb[:, 0:TQ]).astype(_bf16)
        m["xT_hi"] = np.ascontiguousarray(xTb[:, TQ:T]).astype(_bf16)
        m["xT8"] = xTb.astype(_f8)
        m["cosK"] = np.ascontiguousarray(cos[:, colperm]).astype(_bf16)
        m["ssgnK"] = np.ascontiguousarray(ssgn[:, colperm]).astype(_bf16)
        m["mscal"] = np.full((P, 1), 0.0 if h == 0 else 1.0, np.float32)
        in_maps.append(m)
    return in_maps


def gather(results):
    out = np.empty((B, T, H), np.float32)
    for c in range(NCORES):
        b, h = c // 2, c % 2
        out[b, h * TQ:(h + 1) * TQ, :] = results[c]["outT"].T
    return out


# ----------------------------------------------------------------------------
# public entry point
# ----------------------------------------------------------------------------

_NC = None


def zero_flags(in_maps):
    m = in_maps[0]
    amax = lambda k: float(np.abs(np.asarray(m[k], np.float32)).max())
    return dict(
        c0=(amax("cq") == 0.0 and amax("ck") == 0.0),
        bo0=(amax("bo") == 0.0),
        cv0=(amax("cvb") == 0.0),
    )


def kernel(**inputs):
    global _NC
    from concourse.bass_utils import run_bass_kernel_spmd
    in_maps = prepare_in_maps(inputs)
    if _NC is None:
        _NC = build(repeat=1, **zero_flags(in_maps))
    res = run_bass_kernel_spmd(_NC, in_maps, list(range(NCORES)))
    return gather(res.results)
